# revision 3
# baseline (speedup 1.0000x reference)
"""BFP activation quantization kernel for 8 TRN2 NeuronCores.

Reference semantics (per (n,h,w) block over C=256 channels):
    max_abs = max_c |x|
    e such that max_abs = m * 2^e, m in [0.5, 1)   (frexp)
    delta = 2^(e-3)
    q = trunc(x / delta) * delta

Sharding: pure data-parallel over N (batch): 64 images -> 8 per core.

Per-core implementation (layout [c -> partitions, hw -> free], fully
contiguous DMA):
  - gpsimd.partition_all_reduce(absmax) gives per-column |max| over the 128
    partitions, broadcast to all partitions; one vector max merges the two
    C halves.
  - The scale factors are pure exponent-bit arithmetic on int32 views:
        eb = bits(max_abs) & 0x7f800000
        bits(-delta) = eb + 0x7f000000   == int32 value (eb - 0x81000000)
        bits(-recip) = 2^32 - eb         == int32 value (-eb)
    (computed on ScalarE as exact fp32-valued integer ops)
  - trunc(x * recip) with only round-to-nearest-even converters:
        x |= 1            (odd mantissa -> t can never be an exact integer,
                           so the RN tie cases below never bite)
        t  = x * -recip
        sh = sign(t) | bits(0.5 - 2^-25)
        w  = convert_int32(t - sh)       == trunc(t) exactly
        q  = w * -delta
    Verified bit-exact vs the reference on HW over exhaustive mantissa
    sweeps and on-device probes.
"""

import sys

for _p in ("/opt/trn_rl_repo", "/root/.axon_site/_ro/trn_rl_repo"):
    if _p not in sys.path:
        sys.path.append(_p)

import numpy as np

N, C, H, W = 64, 256, 56, 56
HW = H * W  # 3136
NCORES = 8
NPC = N // NCORES  # images per core
F = 1568  # free-dim chunk (hw elements per tile); HW % F == 0
NCHUNK = HW // F


def _i32(v):
    v &= 0xFFFFFFFF
    return v - (1 << 32) if v >= (1 << 31) else v


_cache = {}


def _build():
    if "nc" in _cache:
        return _cache["nc"]

    import concourse.bacc as bacc
    import concourse.mybir as mybir
    import concourse.tile as tile
    from concourse import bass_isa

    dt = mybir.dt
    op = mybir.AluOpType

    nc = bacc.Bacc(
        "TRN2",
        target_bir_lowering=False,
        debug=False,
        enable_asserts=False,
        num_devices=NCORES,
    )
    x_d = nc.dram_tensor("x", [NPC, C, HW], dt.float32, kind="ExternalInput").ap()
    y_d = nc.dram_tensor("y", [NPC, C, HW], dt.float32, kind="ExternalOutput").ap()

    with tile.TileContext(nc) as tc:
        with (
            tc.tile_pool(name="io", bufs=3) as io,
            tc.tile_pool(name="mid", bufs=2) as mid,
            tc.tile_pool(name="consts", bufs=1) as consts,
        ):
            bias = consts.tile([128, 1], dt.float32)
            nc.vector.memset(bias[:], -float(0x81000000))

            for n in range(NPC):
                for j in range(NCHUNK):
                    sl = slice(j * F, (j + 1) * F)

                    xt = io.tile([128, 2 * F], dt.float32, tag="xt")
                    nc.sync.dma_start(out=xt[:, 0:F], in_=x_d[n, 0:128, sl])
                    nc.sync.dma_start(out=xt[:, F : 2 * F], in_=x_d[n, 128:256, sl])

                    # per-column absmax over partitions, broadcast
                    mxt = mid.tile([128, 2 * F], dt.float32, tag="mxt")
                    nc.gpsimd.partition_all_reduce(
                        mxt[:], xt[:], 128, bass_isa.ReduceOp.absmax
                    )
                    # merge c-halves
                    mx = mid.tile([128, F], dt.float32, tag="mx")
                    nc.vector.tensor_tensor(
                        out=mx[:], in0=mxt[:, 0:F], in1=mxt[:, F : 2 * F], op=op.max
                    )
                    # exponent bits
                    eb = mid.tile([128, F], dt.int32, tag="eb")
                    nc.vector.tensor_scalar(
                        out=eb[:], in0=mx[:].bitcast(dt.int32),
                        scalar1=_i32(0x7F800000), scalar2=None, op0=op.bitwise_and,
                    )
                    # nd = bits(-delta), nr = bits(-recip)  (ScalarE)
                    nd = mid.tile([128, F], dt.int32, tag="nd")
                    nr = mid.tile([128, F], dt.int32, tag="nr")
                    nc.scalar.activation(
                        out=nd[:], in_=eb[:],
                        func=mybir.ActivationFunctionType.Identity,
                        bias=bias[:], scale=1.0,
                    )
                    nc.scalar.mul(out=nr[:], in_=eb[:], mul=-1.0)

                    # x |= 1 (in place)
                    nc.vector.tensor_scalar(
                        out=xt[:].bitcast(dt.int32), in0=xt[:].bitcast(dt.int32),
                        scalar1=1, scalar2=None, op0=op.bitwise_or,
                    )
                    # t = x * -recip  (one wide op; nr repeated via stride-0 dim)
                    # reuse mxt slot (dead after merge)
                    tt_ = mxt
                    nrf = nr[:].bitcast(dt.float32)[:, None, :].broadcast_to(
                        [128, 2, F]
                    )
                    x3 = xt[:].rearrange("p (r f) -> p r f", r=2)
                    nc.vector.tensor_tensor(
                        out=tt_[:].rearrange("p (r f) -> p r f", r=2),
                        in0=x3, in1=nrf, op=op.mult,
                    )
                    # sh = sign(t) | bits(0.5 - 2^-25)
                    sh = mid.tile([128, 2 * F], dt.int32, tag="sh")
                    nc.vector.tensor_scalar(
                        out=sh[:], in0=tt_[:].bitcast(dt.int32),
                        scalar1=_i32(0x80000000), scalar2=_i32(0x3EFFFFFF),
                        op0=op.bitwise_and, op1=op.bitwise_or,
                    )
                    # w = RN_int32(t - sh) == trunc(t); in place over tt_
                    w = tt_[:].bitcast(dt.int32)
                    nc.vector.tensor_tensor(
                        out=w, in0=tt_[:], in1=sh[:].bitcast(dt.float32),
                        op=op.subtract,
                    )
                    # q = w * -delta (one wide op); reuse xt slot
                    ndf = nd[:].bitcast(dt.float32)[:, None, :].broadcast_to(
                        [128, 2, F]
                    )
                    nc.vector.tensor_tensor(
                        out=x3, in0=w.rearrange("p (r f) -> p r f", r=2),
                        in1=ndf, op=op.mult,
                    )

                    # stores on the Activation HWDGE ring (separate FIFO
                    # from the SP-ring loads)
                    nc.scalar.dma_start(out=y_d[n, 0:128, sl], in_=xt[:, 0:F])
                    nc.scalar.dma_start(out=y_d[n, 128:256, sl], in_=xt[:, F : 2 * F])

    nc.compile()
    _cache["nc"] = nc
    return nc


def _run(x, trace=False, **kwargs):
    from concourse import bass_utils

    nc = _build()
    xs = np.ascontiguousarray(x.reshape(N, C, HW))
    in_maps = [
        {"x": xs[i * NPC : (i + 1) * NPC]} for i in range(NCORES)
    ]
    res = bass_utils.run_bass_kernel_spmd(
        nc, in_maps, core_ids=list(range(NCORES)), trace=trace, **kwargs
    )
    out = np.concatenate([r["y"] for r in res.results], axis=0)
    return out.reshape(N, C, H, W), res


def kernel(activations):
    out, _ = _run(np.asarray(activations))
    return out
